# revision 3
# baseline (speedup 1.0000x reference)
"""MoE gate routing kernel for Trainium2 (8 NeuronCores).

Computes the DeepSeek-style MoE gate of reference.py:
  gates = x @ W.T ; scores = sigmoid(gates) ; s = scores + bias
  group top-2 sums -> keep top-4 of 8 groups -> top-8 experts of masked s
  sel = normalized unbiased scores * 2.5
Returns (inds int32 [4,4096,8], sel float32 [4,4096,8]).

Sharding: data-parallel over tokens; each of the 8 cores routes 2048 tokens.
Host prep: x is transposed per-core to [H, tokens] and split into fp16
hi/lo pairs (lo scaled by 2^11) so the PE computes fp32-accurate gates with
3 fp16-rate matmul terms: x@W ~= xh@wh + 2^-11*(xh@wl' + xl'@wh),
wl' = (w-wh)*2^11, xl' = (x-xh)*2^11.  hi-term and lo-terms accumulate in
separate PSUM column regions; ScalarE+DVE recombine them.

Startup is DMA-debt-bound (W 4MiB + tile-0 x 2MiB must land before tile 0
can finish): hi/lo matmuls are interleaved per k-chunk to smooth W
consumption, DMAs are emitted in need-order, and a dummy-matmul burst keeps
the PE HAM clock-gate warm through the initial DMA wait.
"""
import numpy as np

B, S, H, E = 4, 4096, 4096, 256
NCORES = 8
T = B * S
TPC = T // NCORES          # tokens per core
PT = 128                   # tokens per tile (partition dim)
TILES = TPC // PT          # 16
KCH = H // 128             # 32 contraction chunks
G = 8                      # expert groups
EG = E // G                # experts per group
TOPK_GROUP = 4
TOP_K = 8
LO_SCALE = 2048.0          # 2^11
ROUTED_SCALING_FACTOR = 2.5
WC = 4                     # k-chunks per W DMA chunk
NWCH = KCH // WC           # 8 W chunks
WARM_MMS = 48

_CACHE = {}


def _build():
    import concourse.tile as tile
    from concourse import bacc, mybir

    F32 = mybir.dt.float32
    F16 = mybir.dt.float16
    U32 = mybir.dt.uint32
    Alu = mybir.AluOpType

    nc = bacc.Bacc(None, target_bir_lowering=False)
    # x hi/lo are pre-permuted on host to per-tile SBUF layout:
    # [TILES*128, KCH*PT] where row = tile*128 + partition, col = k*PT + t
    # wcat is pre-permuted to [128, KCH*2E]: row = h%128, col = (h//128)*2E + e
    xh_d = nc.dram_tensor("xh", [TPC, KCH * PT], F16, kind="ExternalInput")
    xl_d = nc.dram_tensor("xl", [TPC, KCH * PT], F16, kind="ExternalInput")
    w_d = nc.dram_tensor("wcat", [128, KCH * 2 * E], F16, kind="ExternalInput")
    b_d = nc.dram_tensor("bias", [1, E], F32, kind="ExternalInput")
    # outputs in [partition, tile, k] layout (contiguous per partition);
    # host reassembles to token order
    inds_d = nc.dram_tensor("inds", [128, TILES * TOP_K], U32,
                            kind="ExternalOutput")
    sel_d = nc.dram_tensor("sel", [128, TILES * TOP_K], F32,
                           kind="ExternalOutput")

    xh_v = xh_d.rearrange("(n p) f -> n p f", p=128)
    xl_v = xl_d.rearrange("(n p) f -> n p f", p=128)

    with tile.TileContext(nc) as tc:
        with (
            tc.tile_pool(name="const", bufs=1) as cpool,
            tc.tile_pool(name="xload", bufs=4) as xpool,
            tc.tile_pool(name="work", bufs=3) as pool,
            tc.tile_pool(name="out", bufs=1) as opool,
            tc.tile_pool(name="psum", bufs=3, space="PSUM") as pp,
        ):
            # per-chunk W tiles -> fine-grained DMA->matmul dependencies
            w_ch = [cpool.tile([128, WC, 2 * E], F16, tag=f"w{c}",
                               name=f"w{c}")
                    for c in range(NWCH)]

            def load_w_chunk(c):
                nc.sync.dma_start(
                    w_ch[c].rearrange("p k e -> p (k e)"),
                    w_d[:, c * WC * 2 * E:(c + 1) * WC * 2 * E])

            def load_x_part(dst, src_v, tt, lo, hi):
                # load k-chunks [lo, hi) of tile tt
                nc.sync.dma_start(
                    dst[:, lo:hi, :].rearrange("p k t -> p (k t)"),
                    src_v[tt][:, lo * PT:hi * PT])

            xh_t0 = xpool.tile([128, KCH, PT], F16, tag="xh")
            xl_t0 = xpool.tile([128, KCH, PT], F16, tag="xl")

            # DMA emission in need-order: the hi/lo-interleaved k-loop of
            # tile 0 consumes W chunk c and x k-range [8c, 8c+8) together
            load_w_chunk(0)
            QK = 2 * WC  # x k-chunks per quarter
            load_x_part(xh_t0, xh_v, 0, 0, QK)
            load_x_part(xl_t0, xl_v, 0, 0, QK)
            bias_bc = cpool.tile([128, E], F32, tag="bias")
            nc.sync.dma_start(bias_bc, b_d[:].to_broadcast([128, E]))
            load_w_chunk(1)
            load_w_chunk(2)
            load_x_part(xh_t0, xh_v, 0, QK, 2 * QK)
            load_x_part(xl_t0, xl_v, 0, QK, 2 * QK)
            load_w_chunk(3)
            load_w_chunk(4)
            load_x_part(xh_t0, xh_v, 0, 2 * QK, 3 * QK)
            load_x_part(xl_t0, xl_v, 0, 2 * QK, 3 * QK)
            load_w_chunk(5)
            load_w_chunk(6)
            load_x_part(xh_t0, xh_v, 0, 3 * QK, 4 * QK)
            load_x_part(xl_t0, xl_v, 0, 3 * QK, 4 * QK)
            load_w_chunk(7)

            inds_st = opool.tile([128, TILES, TOP_K], U32, tag="inds_st")
            sel_st = opool.tile([128, TILES, TOP_K], F32, tag="sel_st")

            # warm the PE HAM clock-gate during the startup DMA wait: a burst
            # of dummy matmuls sized to end roughly when tile-0 data lands
            warm = cpool.tile([128, 128], F16, tag="warm")
            nc.gpsimd.memset(warm, 0)
            warm_ps = pp.tile([128, 128], F32, tag="warm_ps")
            for _ in range(WARM_MMS):
                nc.tensor.matmul(warm_ps, warm, warm, start=True, stop=True)

            flushed = 0

            def flush_outputs(upto):
                nonlocal flushed
                lo, hi = flushed * TOP_K, upto * TOP_K
                nc.sync.dma_start(
                    inds_d[:, lo:hi],
                    inds_st[:, flushed:upto, :].rearrange("p n k -> p (n k)"))
                nc.sync.dma_start(
                    sel_d[:, lo:hi],
                    sel_st[:, flushed:upto, :].rearrange("p n k -> p (n k)"))
                flushed = upto

            for tt in range(TILES):
                if tt == 0:
                    xh_t, xl_t = xh_t0, xl_t0
                else:
                    xh_t = xpool.tile([128, KCH, PT], F16, tag="xh")
                    xl_t = xpool.tile([128, KCH, PT], F16, tag="xl")
                    HK = KCH // 2
                    load_x_part(xh_t, xh_v, tt, 0, HK)
                    load_x_part(xl_t, xl_v, tt, 0, HK)
                    load_x_part(xh_t, xh_v, tt, HK, KCH)
                    load_x_part(xl_t, xl_v, tt, HK, KCH)

                # pA[:, :E] accumulates xh@wh ; pA[:, E:] accumulates
                # xh@wl' + xl@wh (both lo-terms share the 2^11 scale).
                # hi/lo interleaved per k: W chunk c is consumed over
                # 8*325ns instead of 8*216ns (startup is W-DMA-paced)
                pA = pp.tile([128, 2 * E], F32, tag="pA")
                for k in range(KCH):
                    wk = w_ch[k // WC][:, k % WC, :]
                    nc.tensor.matmul(pA, xh_t[:, k, :], wk,
                                     start=(k == 0), stop=False)
                    nc.tensor.matmul(pA[:, E:], xl_t[:, k, :], wk[:, :E],
                                     start=False, stop=(k == KCH - 1))

                # gates = pA[:, :E] + pA[:, E:] / 2^11
                # (DVE may read only one PSUM operand per op; the scaled
                # copy runs on ScalarE which also reads PSUM)
                tmp = pool.tile([128, E], F32, tag="tmp")
                nc.scalar.mul(tmp, pA[:, E:], 1.0 / LO_SCALE)
                gates = pool.tile([128, E], F32, tag="gates")
                nc.vector.tensor_add(gates, pA[:, :E], tmp)

                # scores = sigmoid(gates); s = scores + bias
                scores = pool.tile([128, E], F32, tag="scores")
                nc.scalar.activation(scores, gates,
                                     mybir.ActivationFunctionType.Sigmoid)
                s = pool.tile([128, E], F32, tag="s")
                nc.vector.tensor_add(s, scores, bias_bc)

                # group scores: top-2 sum per group of 32 via batched
                # reduce-max + match_replace + reduce-max
                s_g = s.rearrange("p (g j) -> p g j", g=G)
                gm1 = pool.tile([128, G], F32, tag="gm1")
                nc.vector.tensor_reduce(gm1, s_g, mybir.AxisListType.X,
                                        Alu.max)
                srep = pool.tile([128, E], F32, tag="srep")
                nc.vector.match_replace(srep, gm1, s, -1.0)
                gm2 = pool.tile([128, G], F32, tag="gm2")
                nc.vector.tensor_reduce(
                    gm2, srep.rearrange("p (g j) -> p g j", g=G),
                    mybir.AxisListType.X, Alu.max)
                gsc = pool.tile([128, G], F32, tag="gsc")
                nc.vector.tensor_add(gsc, gm1, gm2)

                # keep top-4 groups; sm = s where group kept else 0
                gsort = pool.tile([128, 8], F32, tag="gsort")
                nc.vector.max(out=gsort, in_=gsc)
                sm = pool.tile([128, E], F32, tag="sm")
                nc.vector.scalar_tensor_tensor(
                    sm.rearrange("p (g j) -> p g j", g=G),
                    gsc.unsqueeze(2).broadcast_to([128, G, EG]),
                    gsort[:, TOPK_GROUP - 1:TOPK_GROUP],
                    s_g,
                    op0=Alu.is_ge, op1=Alu.mult)

                # top-8 experts by biased score; indices straight into the
                # output staging tile (u32 bits == positive int32)
                vals8 = pool.tile([128, 8], F32, tag="vals8")
                nc.vector.max(out=vals8, in_=sm)
                nc.vector.max_index(inds_st[:, tt, :], vals8, sm)

                # unbiased scores of the selected 8 (unbiased descending
                # order); den = sum of selected scores via fused accumulate
                ssel = pool.tile([128, E], F32, tag="ssel")
                den = pool.tile([128, 1], F32, tag="den")
                nc.vector.scalar_tensor_tensor(
                    ssel, sm, vals8[:, 7:8], scores,
                    op0=Alu.is_ge, op1=Alu.mult, accum_out=den)
                denr = pool.tile([128, 1], F32, tag="denr")
                nc.vector.reciprocal(denr, den)
                u8 = pool.tile([128, 8], F32, tag="u8")
                nc.vector.max(out=u8, in_=ssel)
                idx2 = pool.tile([128, 8], U32, tag="idx2")
                nc.vector.max_index(idx2, u8, ssel)

                # realign unbiased values to biased rank order (8x8 match on
                # raw u32 indices); selr = eq @ u8
                eq3 = pool.tile([128, 8, 8], F32, tag="eq3")
                nc.vector.tensor_tensor(
                    eq3,
                    inds_st[:, tt, :].unsqueeze(2).broadcast_to([128, 8, 8]),
                    idx2.unsqueeze(1).broadcast_to([128, 8, 8]),
                    op=Alu.is_equal)
                prod3 = pool.tile([128, 8, 8], F32, tag="prod3")
                nc.vector.tensor_tensor(
                    prod3, eq3, u8.unsqueeze(1).broadcast_to([128, 8, 8]),
                    op=Alu.mult)
                selr = pool.tile([128, 8], F32, tag="selr")
                nc.vector.reduce_sum(selr, prod3, axis=mybir.AxisListType.X)

                # sel = selr * 2.5 / den   (the reference's +1e-20 guard is
                # dropped: den >= 8 * sigmoid(min gate) >> 0 always)
                nc.vector.scalar_tensor_tensor(
                    sel_st[:, tt, :], selr, ROUTED_SCALING_FACTOR,
                    denr.to_broadcast([128, 8]), op0=Alu.mult, op1=Alu.mult)

                if tt in (3, 7, 11):
                    flush_outputs(tt + 1)

            flush_outputs(TILES)

    nc.compile()
    return nc


def _prep_inputs(x, weight, bias):
    """Host-side shard + transpose + fp16 hi/lo split."""
    xf = np.ascontiguousarray(x.reshape(T, H))
    wT = np.ascontiguousarray(weight.T.astype(np.float32))   # [H, E]
    wh = wT.astype(np.float16)
    wl = ((wT - wh.astype(np.float32)) * LO_SCALE).astype(np.float16)
    wcat = np.concatenate([wh, wl], axis=1)                  # [H, 2E]
    # permute to [128, KCH*2E]: row = h%128, col-major by k-chunk
    wcat = np.ascontiguousarray(
        wcat.reshape(KCH, 128, 2 * E).transpose(1, 0, 2).reshape(128, -1))
    b2 = np.ascontiguousarray(bias.astype(np.float32)[None, :])

    in_maps = []
    for c in range(NCORES):
        xc = xf[c * TPC:(c + 1) * TPC]                     # [TPC, H] f32
        # device layout [tile*128+p, k*PT+t] = x[tile*PT+t, k*128+p]:
        # x^T arranged so each per-tile DMA is contiguous per partition
        xt = xc.T.reshape(KCH, 128, TILES, PT)             # [k, p, tile, t]
        xt = np.ascontiguousarray(xt.transpose(2, 1, 0, 3))  # [tile, p, k, t]
        xt = xt.reshape(TPC, KCH * PT)
        xh = xt.astype(np.float16)
        xl = ((xt - xh.astype(np.float32)) * LO_SCALE).astype(np.float16)
        in_maps.append({"xh": xh, "xl": xl, "wcat": wcat, "bias": b2})
    return in_maps


def kernel(x, weight, bias):
    from concourse.bass_utils import run_bass_kernel_spmd

    if "nc" not in _CACHE:
        _CACHE["nc"] = _build()
    nc = _CACHE["nc"]

    in_maps = _prep_inputs(np.asarray(x), np.asarray(weight), np.asarray(bias))
    res = run_bass_kernel_spmd(nc, in_maps, core_ids=list(range(NCORES)))

    def unpack(a):
        # [128, TILES*TOP_K] -> [TILES*128, TOP_K] token order
        return a.reshape(128, TILES, TOP_K).transpose(1, 0, 2).reshape(
            TPC, TOP_K)

    inds = np.concatenate([unpack(r["inds"]) for r in res.results], axis=0)
    sel = np.concatenate([unpack(r["sel"]) for r in res.results], axis=0)
    return (inds.reshape(B, S, TOP_K).astype(np.int32),
            sel.reshape(B, S, TOP_K).astype(np.float32))
